# revision 6
# baseline (speedup 1.0000x reference)
"""Trainium2 Bass kernel for nn_BottomUpNet (dense_mlp).

Reference computation (per row n of N=8192, fully independent across rows):
    summary = aggregate (broadcast)                   # (1024,)
    for k in 0..15:
        x = [summary, towers[n, k, :]]                # (1088,)
        h = relu(x @ OW1 + Ob1); h = relu(h @ OW2 + Ob2)
        pred_k = sigmoid(h @ OW3 + Ob3)
        m = relu(x @ MW1 + Mb1); m = relu(m @ MW2 + Mb2); m = relu(m @ MW3 + Mb3)
        summary = m
    out[n] = prod_k pred_k

Strategy: data-parallel over N across 8 cores (1024 rows each), weights
replicated.  Activations are feature-major ([feature partition, row free])
so weight matrices serve directly as the stationary matmul operand and no
on-chip transposes are needed.  Matmuls in bf16 with f32 PSUM accumulation
(end-to-end rel err vs the f32 reference ~8e-4); bias+relu epilogues on the
scalar engine out of PSUM.

Perf structure:
  - layer-1 tower matmuls (contraction 64) for the M- and O-branches are
    paired into disjoint PE row groups (0-63 / 64-127) so they run
    concurrently in the systolic array.
  - the 1024->1 output head is computed as a DVE per-partition
    multiply/add tree (g = sum_i h2_i * w3_i) followed by a single
    ones-vector matmul for the cross-partition reduce, instead of eight
    M=1 matmuls.
  - startup DMAs are spread across four DGE queues and ordered by first
    use; the initial summary broadcast is done on-chip from a 4KB vector.
"""

import numpy as np
import ml_dtypes

import concourse.bacc as bacc
import concourse.mybir as mybir
import concourse.tile as tile
from concourse.bass import ts, ds
from concourse.bass_utils import run_bass_kernel_spmd

BF16 = ml_dtypes.bfloat16

N_CORES = 8
N = 8192
K = 16
NI = 64          # tower features per step
NH = 1024        # hidden width
FT = NH // 128   # feature tiles (8)
R = N // N_CORES  # rows per core (1024)
RB = 512         # row block (matmul moving dim / one PSUM bank)
NR = R // RB     # row blocks per core (2)

_BUILT = None


def _build():
    nc = bacc.Bacc("TRN2", target_bir_lowering=False, debug=False,
                   num_devices=N_CORES)
    f32 = mybir.dt.float32
    bf = mybir.dt.bfloat16

    towd = nc.declare_dram_parameter("tow", [K, NI, R], bf, isOutput=False)
    aggd = nc.declare_dram_parameter("agg", [128, FT], f32, isOutput=False)
    mw1sd = nc.declare_dram_parameter("mw1s", [NH, NH], bf, isOutput=False)
    mw1td = nc.declare_dram_parameter("mw1t", [NI, NH], bf, isOutput=False)
    mw2d = nc.declare_dram_parameter("mw2", [NH, NH], bf, isOutput=False)
    mw3d = nc.declare_dram_parameter("mw3", [NH, NH], bf, isOutput=False)
    ow1sd = nc.declare_dram_parameter("ow1s", [NH, NH], bf, isOutput=False)
    ow1td = nc.declare_dram_parameter("ow1t", [NI, NH], bf, isOutput=False)
    ow2d = nc.declare_dram_parameter("ow2", [NH, NH], bf, isOutput=False)
    w3cd = nc.declare_dram_parameter("w3c", [128, FT], f32, isOutput=False)
    balld = nc.declare_dram_parameter("ball", [128, 40], f32, isOutput=False)
    ob3d = nc.declare_dram_parameter("ob3", [1, 1], f32, isOutput=False)
    outd = nc.declare_dram_parameter("out", [1, R], f32, isOutput=True)

    Relu = mybir.ActivationFunctionType.Relu
    Sigmoid = mybir.ActivationFunctionType.Sigmoid
    Add = mybir.AluOpType.add
    Mult = mybir.AluOpType.mult

    with tile.TileContext(nc) as tc:
        with (
            tc.tile_pool(name="weights", bufs=1) as wp,
            tc.tile_pool(name="summary", bufs=1) as sp,
            tc.tile_pool(name="acts", bufs=16) as ap,
            tc.tile_pool(name="tow", bufs=4) as twp,
            tc.tile_pool(name="small", bufs=1) as smp,
            tc.tile_pool(name="zwork", bufs=2) as zw,
            tc.tile_pool(name="psum", bufs=6, space="PSUM") as pp,
            tc.tile_pool(name="zpsum", bufs=2, space="PSUM") as zp,
        ):
            # --- weights, spread across DGE queues by first use ---
            def load_w(dram, name, eng):
                tiles = []
                for i in range(FT):
                    t = wp.tile([128, NH], bf, tag=f"{name}{i}",
                                name=f"{name}{i}")
                    eng.dma_start(out=t, in_=dram[ts(i, 128), :])
                    tiles.append(t)
                return tiles

            # Step 0 runs the M branch first (layer1 unfused), so only
            # mw1s/mw1t gate the start.  Split mw1s across the two HW DGE
            # queues; everything else ordered by first use.
            def load_w_split(dram, name, engs):
                tiles = []
                for i in range(FT):
                    t = wp.tile([128, NH], bf, tag=f"{name}{i}",
                                name=f"{name}{i}")
                    engs[i % len(engs)].dma_start(out=t, in_=dram[ts(i, 128), :])
                    tiles.append(t)
                return tiles

            ball = smp.tile([128, 40], f32, tag="ball", name="ball")
            nc.scalar.dma_start(out=ball, in_=balld[:])
            ob3 = smp.tile([1, 1], f32, tag="ob3", name="ob3")
            nc.scalar.dma_start(out=ob3, in_=ob3d[:])
            aggt = smp.tile([128, FT], f32, tag="aggt", name="aggt")
            nc.scalar.dma_start(out=aggt, in_=aggd[:])
            w3c = smp.tile([128, FT], f32, tag="w3c", name="w3c")
            nc.scalar.dma_start(out=w3c, in_=w3cd[:])
            mw1s = load_w_split(mw1sd, "mw1s", [nc.sync, nc.scalar])
            mw1t = wp.tile([NI, NH], bf, tag="mw1t", name="mw1t")
            nc.gpsimd.dma_start(out=mw1t, in_=mw1td[:])
            ow1t = wp.tile([128, NH], bf, tag="ow1t", name="ow1t")
            nc.gpsimd.memset(ow1t, 0.0)
            nc.sync.dma_start(out=ow1t[64:128, :], in_=ow1td[:])
            ow1s = load_w_split(ow1sd, "ow1s", [nc.sync, nc.scalar])
            # gpsimd (SW DGE): layer 2/3 weights, needed tens of us in
            mw2 = load_w(mw2d, "mw2", nc.gpsimd)
            mw3 = load_w(mw3d, "mw3", nc.gpsimd)
            ow2 = load_w(ow2d, "ow2", nc.gpsimd)

            ones = smp.tile([128, 1], bf, tag="ones", name="ones")
            nc.vector.memset(ones, 1.0)
            zero = smp.tile([128, RB], bf, tag="zero", name="zero")
            nc.gpsimd.memset(zero, 0.0)

            # --- summary double buffer, init = broadcast of aggregate ---
            sA = [[sp.tile([128, RB], bf, tag=f"sA{i}_{r}",
                           name=f"sA{i}_{r}") for r in range(NR)]
                  for i in range(FT)]
            sB = [[sp.tile([128, RB], bf, tag=f"sB{i}_{r}",
                           name=f"sB{i}_{r}") for r in range(NR)]
                  for i in range(FT)]
            for r in range(NR):          # r0 first: layer1 r0 starts sooner
                for i in range(FT):
                    nc.vector.tensor_scalar(
                        sA[i][r][:], zero[:], aggt[:, ds(i, 1)], None, Add)

            # --- product accumulators ---
            pacc = []
            for r in range(NR):
                t = smp.tile([1, RB], f32, tag=f"pacc{r}", name=f"pacc{r}")
                nc.vector.memset(t, 1.0)
                pacc.append(t)

            # bias column index per layer: 0=Mb1 1=Mb2 2=Mb3 3=Ob1 4=Ob2
            def relu_epilogue(ot, ps, bias_l, m):
                """Bias+relu out of PSUM; alternate ACT/DVE by m so neither
                engine head-of-line-blocks the PE's psum bank rotation."""
                bias = ball[:, ds(bias_l * 8 + m, 1)]
                if m % 2 == 0:
                    nc.scalar.activation(ot[:], ps[:], Relu, bias=bias)
                else:
                    nc.vector.tensor_scalar(ot[:], ps[:], bias, 0.0, Add,
                                            mybir.AluOpType.max)

            def layer1(scur, tow_t, branches=("mo",)):
                """Fused M/O layer 1.  Per (r, m): the M accumulation group
                ends with the contraction-64 tower matmul on PE rows 0-63,
                and the O group begins with its tower matmul on rows 64-127
                so the two stream concurrently in the array.  branches
                allows the unfused ("m",), ("o",) split for step 0."""
                m1o = [[None] * FT for _ in range(NR)]
                h1o = [[None] * FT for _ in range(NR)]
                for br in branches:
                    for r in range(NR):
                        for m in range(FT):
                            if "m" in br:
                                psm = pp.tile([128, RB], mybir.dt.float32,
                                              tag="ps", name="psm")
                                for i in range(FT):
                                    nc.tensor.matmul(
                                        psm[:], mw1s[i][:, ts(m, 128)],
                                        scur[i][r][:],
                                        start=(i == 0), stop=False)
                                nc.tensor.matmul(
                                    psm[:], mw1t[:, ts(m, 128)],
                                    tow_t[0:NI, ts(r, RB)],
                                    start=False, stop=True)
                            if "o" in br:
                                pso = pp.tile([128, RB], mybir.dt.float32,
                                              tag="ps", name="pso")
                                nc.tensor.matmul(
                                    pso[:], ow1t[64:128, ts(m, 128)],
                                    tow_t[64:128, ts(r, RB)],
                                    start=True, stop=False)
                                for i in range(FT):
                                    nc.tensor.matmul(
                                        pso[:], ow1s[i][:, ts(m, 128)],
                                        scur[i][r][:],
                                        start=False, stop=(i == FT - 1))
                            if "m" in br:
                                m1t = ap.tile([128, RB], bf, tag="m1",
                                              name="m1")
                                relu_epilogue(m1t, psm, 0, m)
                                m1o[r][m] = m1t
                            if "o" in br:
                                h1t = ap.tile([128, RB], bf, tag="h1",
                                              name="h1")
                                relu_epilogue(h1t, pso, 3, m)
                                h1o[r][m] = h1t
                reidx = lambda o: [[o[r][m] for r in range(NR)]
                                   for m in range(FT)]
                return reidx(m1o), reidx(h1o)

            def layer(rhs, ws, bias_l, out_tag, out_tiles=None):
                outs = []
                for r in range(NR):
                    row = []
                    for m in range(FT):
                        ps = pp.tile([128, RB], mybir.dt.float32, tag="ps",
                                     name="ps")
                        for i in range(FT):
                            nc.tensor.matmul(
                                ps[:], ws[i][:, ts(m, 128)], rhs[i][r][:],
                                start=(i == 0), stop=(i == FT - 1))
                        if out_tiles is not None:
                            ot = out_tiles[m][r]
                        else:
                            ot = ap.tile([128, RB], bf, tag=out_tag,
                                         name=out_tag)
                        relu_epilogue(ot, ps, bias_l, m)
                        row.append(ot)
                    outs.append(row)
                return [[outs[r][m] for r in range(NR)] for m in range(FT)]

            def flush_zjobs(zjobs):
                for gb, r in zjobs:
                    zps = zp.tile([1, RB], mybir.dt.float32, tag="z",
                                  name="zps")
                    nc.tensor.matmul(zps[:], ones[:], gb[:],
                                     start=True, stop=True)
                    pr = smp.tile([1, RB], mybir.dt.float32, tag=f"pr{r}",
                                  name=f"pr{r}")
                    nc.scalar.activation(pr[:], zps[:], Sigmoid, bias=ob3[:])
                    nc.vector.tensor_mul(pacc[r][:], pacc[r][:], pr[:])

            scur, snxt = sA, sB
            zjobs = []
            for k in range(K):
                tow_t = twp.tile([128, R], bf, tag="tow", name="tow")
                nc.scalar.dma_start(out=tow_t[0:NI, :], in_=towd[k])
                nc.scalar.dma_start(out=tow_t[64:128, :], in_=towd[k])

                m1, h1 = layer1(scur, tow_t,
                                branches=("m", "o") if k == 0 else ("mo",))
                m2 = layer(m1, mw2, 1, "l2")
                # previous step's output head (its DVE reduce is long done,
                # so the sigmoid never head-of-line-blocks the ACT queue)
                flush_zjobs(zjobs)
                zjobs = []
                layer(m2, mw3, 2, None, out_tiles=snxt)
                h2 = layer(h1, ow2, 4, "l2")
                # g = sum_i h2_i * w3_i on the DVE (per-partition scalars),
                # reduced across partitions next step by a ones-matmul.
                for r in range(NR):
                    g = zw.tile([128, RB], mybir.dt.float32, tag="g",
                                name="g")
                    nc.vector.tensor_scalar(
                        g[:], h2[0][r][:], w3c[:, ds(0, 1)], None, Mult)
                    for i in range(1, FT):
                        t = zw.tile([128, RB], mybir.dt.float32, tag="t",
                                    name="t")
                        nc.vector.tensor_scalar(
                            t[:], h2[i][r][:], w3c[:, ds(i, 1)], None, Mult)
                        nc.vector.tensor_tensor(g[:], g[:], t[:], Add)
                    gb = zw.tile([128, RB], bf, tag="gb", name="gb", bufs=4)
                    nc.vector.tensor_copy(gb[:], g[:])
                    zjobs.append((gb, r))

                scur, snxt = snxt, scur
            flush_zjobs(zjobs)

            for r in range(NR):
                nc.sync.dma_start(out=outd[:, ts(r, RB)], in_=pacc[r][:])

    nc.finalize()
    return nc


def _get_nc():
    global _BUILT
    if _BUILT is None:
        _BUILT = _build()
    return _BUILT


def _prep_inputs(inputs):
    f32 = np.float32
    towers = np.asarray(inputs["towers"], dtype=f32)
    agg = np.asarray(inputs["aggregate"], dtype=f32)
    MW1 = np.asarray(inputs["MW1"], dtype=f32)
    OW1 = np.asarray(inputs["OW1"], dtype=f32)

    shared = {
        "agg": np.ascontiguousarray(agg.reshape(FT, 128).T),
        "mw1s": MW1[:NH].astype(BF16),
        "mw1t": np.ascontiguousarray(MW1[NH:]).astype(BF16),
        "mw2": np.asarray(inputs["MW2"], f32).astype(BF16),
        "mw3": np.asarray(inputs["MW3"], f32).astype(BF16),
        "ow1s": OW1[:NH].astype(BF16),
        "ow1t": np.ascontiguousarray(OW1[NH:]).astype(BF16),
        "ow2": np.asarray(inputs["OW2"], f32).astype(BF16),
        "w3c": np.ascontiguousarray(
            np.asarray(inputs["OW3"], f32).reshape(FT, 128).T),
        "ball": np.concatenate(
            [np.asarray(inputs[b], f32).reshape(FT, 128).T
             for b in ("Mb1", "Mb2", "Mb3", "Ob1", "Ob2")], axis=1),
        "ob3": np.asarray(inputs["Ob3"], f32).reshape(1, 1),
    }
    in_maps = []
    for c in range(N_CORES):
        tc_ = towers[c * R:(c + 1) * R]          # (R, K, NI)
        towT = np.ascontiguousarray(tc_.transpose(1, 2, 0)).astype(BF16)
        in_maps.append({"tow": towT, **shared})
    return in_maps


def _run(inputs, trace=False):
    nc = _get_nc()
    in_maps = _prep_inputs(inputs)
    res = run_bass_kernel_spmd(nc, in_maps, list(range(N_CORES)), trace=trace)
    out = np.concatenate([res.results[c]["out"][0] for c in range(N_CORES)])
    return out.astype(np.float32), res


def kernel(**inputs):
    out, _ = _run(inputs, trace=False)
    return out


# revision 7
# speedup vs baseline: 1.0047x; 1.0047x over previous
"""Trainium2 Bass kernel for nn_BottomUpNet (dense_mlp).

Reference computation (per row n of N=8192, fully independent across rows):
    summary = aggregate (broadcast)                   # (1024,)
    for k in 0..15:
        x = [summary, towers[n, k, :]]                # (1088,)
        h = relu(x @ OW1 + Ob1); h = relu(h @ OW2 + Ob2)
        pred_k = sigmoid(h @ OW3 + Ob3)
        m = relu(x @ MW1 + Mb1); m = relu(m @ MW2 + Mb2); m = relu(m @ MW3 + Mb3)
        summary = m
    out[n] = prod_k pred_k

Strategy: data-parallel over N across 8 cores (1024 rows each), weights
replicated.  Activations are feature-major ([feature partition, row free])
so weight matrices serve directly as the stationary matmul operand and no
on-chip transposes are needed.  Matmuls in bf16 with f32 PSUM accumulation
(end-to-end rel err vs the f32 reference ~8e-4); bias+relu epilogues on the
scalar engine out of PSUM.

Perf structure:
  - layer-1 tower matmuls (contraction 64) for the M- and O-branches are
    paired into disjoint PE row groups (0-63 / 64-127) so they run
    concurrently in the systolic array.
  - the 1024->1 output head is computed as a DVE per-partition
    multiply/add tree (g = sum_i h2_i * w3_i) followed by a single
    ones-vector matmul for the cross-partition reduce, instead of eight
    M=1 matmuls.
  - startup DMAs are spread across four DGE queues and ordered by first
    use; the initial summary broadcast is done on-chip from a 4KB vector.
"""

import numpy as np
import ml_dtypes

import concourse.bacc as bacc
import concourse.mybir as mybir
import concourse.tile as tile
from concourse.bass import ts, ds
from concourse.bass_utils import run_bass_kernel_spmd

BF16 = ml_dtypes.bfloat16

N_CORES = 8
N = 8192
K = 16
NI = 64          # tower features per step
NH = 1024        # hidden width
FT = NH // 128   # feature tiles (8)
R = N // N_CORES  # rows per core (1024)
RB = 512         # row block (matmul moving dim / one PSUM bank)
NR = R // RB     # row blocks per core (2)

_BUILT = None


def _build():
    nc = bacc.Bacc("TRN2", target_bir_lowering=False, debug=False,
                   num_devices=N_CORES)
    f32 = mybir.dt.float32
    bf = mybir.dt.bfloat16

    towd = nc.declare_dram_parameter("tow", [K, NI, R], bf, isOutput=False)
    aggd = nc.declare_dram_parameter("agg", [128, FT], f32, isOutput=False)
    mw1sd = nc.declare_dram_parameter("mw1s", [NH, NH], bf, isOutput=False)
    mw1td = nc.declare_dram_parameter("mw1t", [NI, NH], bf, isOutput=False)
    mw2d = nc.declare_dram_parameter("mw2", [NH, NH], bf, isOutput=False)
    mw3d = nc.declare_dram_parameter("mw3", [NH, NH], bf, isOutput=False)
    ow1sd = nc.declare_dram_parameter("ow1s", [NH, NH], bf, isOutput=False)
    ow1td = nc.declare_dram_parameter("ow1t", [NI, NH], bf, isOutput=False)
    ow2d = nc.declare_dram_parameter("ow2", [NH, NH], bf, isOutput=False)
    w3cd = nc.declare_dram_parameter("w3c", [128, FT], f32, isOutput=False)
    balld = nc.declare_dram_parameter("ball", [128, 40], f32, isOutput=False)
    ob3d = nc.declare_dram_parameter("ob3", [1, 1], f32, isOutput=False)
    outd = nc.declare_dram_parameter("out", [1, R], f32, isOutput=True)

    Relu = mybir.ActivationFunctionType.Relu
    Sigmoid = mybir.ActivationFunctionType.Sigmoid
    Add = mybir.AluOpType.add
    Mult = mybir.AluOpType.mult

    with tile.TileContext(nc) as tc:
        with (
            tc.tile_pool(name="weights", bufs=1) as wp,
            tc.tile_pool(name="summary", bufs=1) as sp,
            tc.tile_pool(name="acts", bufs=16) as ap,
            tc.tile_pool(name="tow", bufs=4) as twp,
            tc.tile_pool(name="small", bufs=1) as smp,
            tc.tile_pool(name="zwork", bufs=2) as zw,
            tc.tile_pool(name="psum", bufs=6, space="PSUM") as pp,
            tc.tile_pool(name="zpsum", bufs=2, space="PSUM") as zp,
        ):
            # --- weights, spread across DGE queues by first use ---
            def load_w(dram, name, eng):
                tiles = []
                for i in range(FT):
                    t = wp.tile([128, NH], bf, tag=f"{name}{i}",
                                name=f"{name}{i}")
                    eng.dma_start(out=t, in_=dram[ts(i, 128), :])
                    tiles.append(t)
                return tiles

            # Step 0 runs the M branch first (layer1 unfused), so only
            # mw1s/mw1t gate the start.  Split mw1s across the two HW DGE
            # queues; everything else ordered by first use.
            def load_w_split(dram, name, engs):
                tiles = []
                for i in range(FT):
                    t = wp.tile([128, NH], bf, tag=f"{name}{i}",
                                name=f"{name}{i}")
                    engs[i % len(engs)].dma_start(out=t, in_=dram[ts(i, 128), :])
                    tiles.append(t)
                return tiles

            # The two HW DGE queues carry all big weights, strictly ordered
            # by first use (layer1 M, layer1 O, layer2, layer3) so the
            # first-needed bytes get the full HBM read bandwidth.  The
            # gpsimd SW queue carries only small early tiles and the
            # per-step tower stream.
            ball = smp.tile([128, 40], f32, tag="ball", name="ball")
            nc.gpsimd.dma_start(out=ball, in_=balld[:])
            ob3 = smp.tile([1, 1], f32, tag="ob3", name="ob3")
            nc.gpsimd.dma_start(out=ob3, in_=ob3d[:])
            aggt = smp.tile([128, FT], f32, tag="aggt", name="aggt")
            nc.gpsimd.dma_start(out=aggt, in_=aggd[:])
            w3c = smp.tile([128, FT], f32, tag="w3c", name="w3c")
            nc.gpsimd.dma_start(out=w3c, in_=w3cd[:])
            mw1t = wp.tile([NI, NH], bf, tag="mw1t", name="mw1t")
            nc.gpsimd.dma_start(out=mw1t, in_=mw1td[:])
            ow1t = wp.tile([128, NH], bf, tag="ow1t", name="ow1t")
            nc.gpsimd.memset(ow1t, 0.0)
            nc.gpsimd.dma_start(out=ow1t[64:128, :], in_=ow1td[:])
            mw1s = load_w_split(mw1sd, "mw1s", [nc.sync, nc.scalar])
            ow1s = load_w_split(ow1sd, "ow1s", [nc.sync, nc.scalar])
            mw2 = load_w_split(mw2d, "mw2", [nc.sync, nc.scalar])
            mw3 = load_w_split(mw3d, "mw3", [nc.sync, nc.scalar])
            ow2 = load_w_split(ow2d, "ow2", [nc.sync, nc.scalar])

            ones = smp.tile([128, 1], bf, tag="ones", name="ones")
            nc.vector.memset(ones, 1.0)
            zero = smp.tile([128, RB], bf, tag="zero", name="zero")
            nc.gpsimd.memset(zero, 0.0)

            # --- summary double buffer, init = broadcast of aggregate ---
            sA = [[sp.tile([128, RB], bf, tag=f"sA{i}_{r}",
                           name=f"sA{i}_{r}") for r in range(NR)]
                  for i in range(FT)]
            sB = [[sp.tile([128, RB], bf, tag=f"sB{i}_{r}",
                           name=f"sB{i}_{r}") for r in range(NR)]
                  for i in range(FT)]
            for r in range(NR):          # r0 first: layer1 r0 starts sooner
                for i in range(FT):
                    nc.vector.tensor_scalar(
                        sA[i][r][:], zero[:], aggt[:, ds(i, 1)], None, Add)

            # --- product accumulators ---
            pacc = []
            for r in range(NR):
                t = smp.tile([1, RB], f32, tag=f"pacc{r}", name=f"pacc{r}")
                nc.vector.memset(t, 1.0)
                pacc.append(t)

            # bias column index per layer: 0=Mb1 1=Mb2 2=Mb3 3=Ob1 4=Ob2
            def relu_epilogue(ot, ps, bias_l, m):
                """Bias+relu out of PSUM; alternate ACT/DVE by m so neither
                engine head-of-line-blocks the PE's psum bank rotation."""
                bias = ball[:, ds(bias_l * 8 + m, 1)]
                if m % 2 == 0:
                    nc.scalar.activation(ot[:], ps[:], Relu, bias=bias)
                else:
                    nc.vector.tensor_scalar(ot[:], ps[:], bias, 0.0, Add,
                                            mybir.AluOpType.max)

            def layer1(scur, tow_t, branches=("mo",)):
                """Fused M/O layer 1.  Per (r, m): the M accumulation group
                ends with the contraction-64 tower matmul on PE rows 0-63,
                and the O group begins with its tower matmul on rows 64-127
                so the two stream concurrently in the array.  branches
                allows the unfused ("m",), ("o",) split for step 0."""
                m1o = [[None] * FT for _ in range(NR)]
                h1o = [[None] * FT for _ in range(NR)]
                for br in branches:
                    for r in range(NR):
                        for m in range(FT):
                            if "m" in br:
                                psm = pp.tile([128, RB], mybir.dt.float32,
                                              tag="ps", name="psm")
                                for i in range(FT):
                                    nc.tensor.matmul(
                                        psm[:], mw1s[i][:, ts(m, 128)],
                                        scur[i][r][:],
                                        start=(i == 0), stop=False)
                                nc.tensor.matmul(
                                    psm[:], mw1t[:, ts(m, 128)],
                                    tow_t[0:NI, ts(r, RB)],
                                    start=False, stop=True)
                            if "o" in br:
                                pso = pp.tile([128, RB], mybir.dt.float32,
                                              tag="ps", name="pso")
                                nc.tensor.matmul(
                                    pso[:], ow1t[64:128, ts(m, 128)],
                                    tow_t[64:128, ts(r, RB)],
                                    start=True, stop=False)
                                for i in range(FT):
                                    nc.tensor.matmul(
                                        pso[:], ow1s[i][:, ts(m, 128)],
                                        scur[i][r][:],
                                        start=False, stop=(i == FT - 1))
                            if "m" in br:
                                m1t = ap.tile([128, RB], bf, tag="m1",
                                              name="m1")
                                relu_epilogue(m1t, psm, 0, m)
                                m1o[r][m] = m1t
                            if "o" in br:
                                h1t = ap.tile([128, RB], bf, tag="h1",
                                              name="h1")
                                relu_epilogue(h1t, pso, 3, m)
                                h1o[r][m] = h1t
                reidx = lambda o: [[o[r][m] for r in range(NR)]
                                   for m in range(FT)]
                return reidx(m1o), reidx(h1o)

            def layer(rhs, ws, bias_l, out_tag, out_tiles=None):
                outs = []
                for r in range(NR):
                    row = []
                    for m in range(FT):
                        ps = pp.tile([128, RB], mybir.dt.float32, tag="ps",
                                     name="ps")
                        for i in range(FT):
                            nc.tensor.matmul(
                                ps[:], ws[i][:, ts(m, 128)], rhs[i][r][:],
                                start=(i == 0), stop=(i == FT - 1))
                        if out_tiles is not None:
                            ot = out_tiles[m][r]
                        else:
                            ot = ap.tile([128, RB], bf, tag=out_tag,
                                         name=out_tag)
                        relu_epilogue(ot, ps, bias_l, m)
                        row.append(ot)
                    outs.append(row)
                return [[outs[r][m] for r in range(NR)] for m in range(FT)]

            def flush_zjobs(zjobs):
                for gb, r in zjobs:
                    zps = zp.tile([1, RB], mybir.dt.float32, tag="z",
                                  name="zps")
                    nc.tensor.matmul(zps[:], ones[:], gb[:],
                                     start=True, stop=True)
                    pr = smp.tile([1, RB], mybir.dt.float32, tag=f"pr{r}",
                                  name=f"pr{r}")
                    nc.scalar.activation(pr[:], zps[:], Sigmoid, bias=ob3[:])
                    nc.vector.tensor_mul(pacc[r][:], pacc[r][:], pr[:])

            scur, snxt = sA, sB
            zjobs = []
            for k in range(K):
                tow_t = twp.tile([128, R], bf, tag="tow", name="tow")
                nc.gpsimd.dma_start(out=tow_t[0:NI, :], in_=towd[k])
                nc.gpsimd.dma_start(out=tow_t[64:128, :], in_=towd[k])

                m1, h1 = layer1(scur, tow_t,
                                branches=("m", "o") if k == 0 else ("mo",))
                m2 = layer(m1, mw2, 1, "l2")
                # previous step's output head (its DVE reduce is long done,
                # so the sigmoid never head-of-line-blocks the ACT queue)
                flush_zjobs(zjobs)
                zjobs = []
                layer(m2, mw3, 2, None, out_tiles=snxt)
                h2 = layer(h1, ow2, 4, "l2")
                # g = sum_i h2_i * w3_i on the DVE (per-partition scalars),
                # reduced across partitions next step by a ones-matmul.
                for r in range(NR):
                    g = zw.tile([128, RB], mybir.dt.float32, tag="g",
                                name="g")
                    nc.vector.tensor_scalar(
                        g[:], h2[0][r][:], w3c[:, ds(0, 1)], None, Mult)
                    for i in range(1, FT):
                        t = zw.tile([128, RB], mybir.dt.float32, tag="t",
                                    name="t")
                        nc.vector.tensor_scalar(
                            t[:], h2[i][r][:], w3c[:, ds(i, 1)], None, Mult)
                        nc.vector.tensor_tensor(g[:], g[:], t[:], Add)
                    gb = zw.tile([128, RB], bf, tag="gb", name="gb", bufs=4)
                    nc.vector.tensor_copy(gb[:], g[:])
                    zjobs.append((gb, r))

                scur, snxt = snxt, scur
            flush_zjobs(zjobs)

            for r in range(NR):
                nc.sync.dma_start(out=outd[:, ts(r, RB)], in_=pacc[r][:])

    nc.finalize()
    return nc


def _get_nc():
    global _BUILT
    if _BUILT is None:
        _BUILT = _build()
    return _BUILT


def _prep_inputs(inputs):
    f32 = np.float32
    towers = np.asarray(inputs["towers"], dtype=f32)
    agg = np.asarray(inputs["aggregate"], dtype=f32)
    MW1 = np.asarray(inputs["MW1"], dtype=f32)
    OW1 = np.asarray(inputs["OW1"], dtype=f32)

    shared = {
        "agg": np.ascontiguousarray(agg.reshape(FT, 128).T),
        "mw1s": MW1[:NH].astype(BF16),
        "mw1t": np.ascontiguousarray(MW1[NH:]).astype(BF16),
        "mw2": np.asarray(inputs["MW2"], f32).astype(BF16),
        "mw3": np.asarray(inputs["MW3"], f32).astype(BF16),
        "ow1s": OW1[:NH].astype(BF16),
        "ow1t": np.ascontiguousarray(OW1[NH:]).astype(BF16),
        "ow2": np.asarray(inputs["OW2"], f32).astype(BF16),
        "w3c": np.ascontiguousarray(
            np.asarray(inputs["OW3"], f32).reshape(FT, 128).T),
        "ball": np.concatenate(
            [np.asarray(inputs[b], f32).reshape(FT, 128).T
             for b in ("Mb1", "Mb2", "Mb3", "Ob1", "Ob2")], axis=1),
        "ob3": np.asarray(inputs["Ob3"], f32).reshape(1, 1),
    }
    in_maps = []
    for c in range(N_CORES):
        tc_ = towers[c * R:(c + 1) * R]          # (R, K, NI)
        towT = np.ascontiguousarray(tc_.transpose(1, 2, 0)).astype(BF16)
        in_maps.append({"tow": towT, **shared})
    return in_maps


def _run(inputs, trace=False):
    nc = _get_nc()
    in_maps = _prep_inputs(inputs)
    res = run_bass_kernel_spmd(nc, in_maps, list(range(N_CORES)), trace=trace)
    out = np.concatenate([res.results[c]["out"][0] for c in range(N_CORES)])
    return out.astype(np.float32), res


def kernel(**inputs):
    out, _ = _run(inputs, trace=False)
    return out


# revision 8
# speedup vs baseline: 1.0172x; 1.0125x over previous
"""Trainium2 Bass kernel for nn_BottomUpNet (dense_mlp).

Reference computation (per row n of N=8192, fully independent across rows):
    summary = aggregate (broadcast)                   # (1024,)
    for k in 0..15:
        x = [summary, towers[n, k, :]]                # (1088,)
        h = relu(x @ OW1 + Ob1); h = relu(h @ OW2 + Ob2)
        pred_k = sigmoid(h @ OW3 + Ob3)
        m = relu(x @ MW1 + Mb1); m = relu(m @ MW2 + Mb2); m = relu(m @ MW3 + Mb3)
        summary = m
    out[n] = prod_k pred_k

Strategy: data-parallel over N across 8 cores (1024 rows each), weights
replicated.  Activations are feature-major ([feature partition, row free])
so weight matrices serve directly as the stationary matmul operand and no
on-chip transposes are needed.  Matmuls in bf16 with f32 PSUM accumulation
(end-to-end rel err vs the f32 reference ~8e-4); bias+relu epilogues on the
scalar engine out of PSUM.

Perf structure:
  - layer-1 tower matmuls (contraction 64) for the M- and O-branches are
    paired into disjoint PE row groups (0-63 / 64-127) so they run
    concurrently in the systolic array.
  - the 1024->1 output head is computed as a DVE per-partition
    multiply/add tree (g = sum_i h2_i * w3_i) followed by a single
    ones-vector matmul for the cross-partition reduce, instead of eight
    M=1 matmuls.
  - startup DMAs are spread across four DGE queues and ordered by first
    use; the initial summary broadcast is done on-chip from a 4KB vector.
"""

import numpy as np
import ml_dtypes

import concourse.bacc as bacc
import concourse.mybir as mybir
import concourse.tile as tile
from concourse.bass import ts, ds
from concourse.bass_utils import run_bass_kernel_spmd

BF16 = ml_dtypes.bfloat16

N_CORES = 8
N = 8192
K = 16
NI = 64          # tower features per step
NH = 1024        # hidden width
FT = NH // 128   # feature tiles (8)
R = N // N_CORES  # rows per core (1024)
RB = 512         # row block (matmul moving dim / one PSUM bank)
NR = R // RB     # row blocks per core (2)

_BUILT = None


def _build():
    nc = bacc.Bacc("TRN2", target_bir_lowering=False, debug=False,
                   num_devices=N_CORES)
    f32 = mybir.dt.float32
    bf = mybir.dt.bfloat16

    towd = nc.declare_dram_parameter("tow", [K, NI, R], bf, isOutput=False)
    aggd = nc.declare_dram_parameter("agg", [128, FT], f32, isOutput=False)
    mw1sd = nc.declare_dram_parameter("mw1s", [NH, NH], bf, isOutput=False)
    mw1td = nc.declare_dram_parameter("mw1t", [NI, NH], bf, isOutput=False)
    mw2d = nc.declare_dram_parameter("mw2", [NH, NH], bf, isOutput=False)
    mw3d = nc.declare_dram_parameter("mw3", [NH, NH], bf, isOutput=False)
    ow1sd = nc.declare_dram_parameter("ow1s", [NH, NH], bf, isOutput=False)
    ow1td = nc.declare_dram_parameter("ow1t", [NI, NH], bf, isOutput=False)
    ow2d = nc.declare_dram_parameter("ow2", [NH, NH], bf, isOutput=False)
    w3cd = nc.declare_dram_parameter("w3c", [128, FT], f32, isOutput=False)
    balld = nc.declare_dram_parameter("ball", [128, 40], f32, isOutput=False)
    ob3d = nc.declare_dram_parameter("ob3", [1, 1], f32, isOutput=False)
    outd = nc.declare_dram_parameter("out", [1, R], f32, isOutput=True)

    Relu = mybir.ActivationFunctionType.Relu
    Sigmoid = mybir.ActivationFunctionType.Sigmoid
    Add = mybir.AluOpType.add
    Mult = mybir.AluOpType.mult

    with tile.TileContext(nc) as tc:
        with (
            tc.tile_pool(name="weights", bufs=1) as wp,
            tc.tile_pool(name="summary", bufs=1) as sp,
            tc.tile_pool(name="acts", bufs=16) as ap,
            tc.tile_pool(name="tow", bufs=4) as twp,
            tc.tile_pool(name="small", bufs=1) as smp,
            tc.tile_pool(name="zwork", bufs=2) as zw,
            tc.tile_pool(name="psum", bufs=6, space="PSUM") as pp,
            tc.tile_pool(name="zpsum", bufs=2, space="PSUM") as zp,
        ):
            # --- weights, spread across DGE queues by first use ---
            def load_w(dram, name, eng):
                tiles = []
                for i in range(FT):
                    t = wp.tile([128, NH], bf, tag=f"{name}{i}",
                                name=f"{name}{i}")
                    eng.dma_start(out=t, in_=dram[ts(i, 128), :])
                    tiles.append(t)
                return tiles

            # Step 0 runs the M branch first (layer1 unfused), so only
            # mw1s/mw1t gate the start.  Split mw1s across the two HW DGE
            # queues; everything else ordered by first use.
            def load_w_split(dram, name, engs):
                tiles = []
                for i in range(FT):
                    t = wp.tile([128, NH], bf, tag=f"{name}{i}",
                                name=f"{name}{i}")
                    engs[i % len(engs)].dma_start(out=t, in_=dram[ts(i, 128), :])
                    tiles.append(t)
                return tiles

            # The two HW DGE queues carry all big weights, strictly ordered
            # by first use (layer1 M, layer1 O, layer2, layer3) so the
            # first-needed bytes get the full HBM read bandwidth.  The
            # gpsimd SW queue carries only small early tiles and the
            # per-step tower stream.
            ball = smp.tile([128, 40], f32, tag="ball", name="ball")
            nc.gpsimd.dma_start(out=ball, in_=balld[:])
            ob3 = smp.tile([1, 1], f32, tag="ob3", name="ob3")
            nc.gpsimd.dma_start(out=ob3, in_=ob3d[:])
            aggt = smp.tile([128, FT], f32, tag="aggt", name="aggt")
            nc.gpsimd.dma_start(out=aggt, in_=aggd[:])
            w3c = smp.tile([128, FT], f32, tag="w3c", name="w3c")
            nc.gpsimd.dma_start(out=w3c, in_=w3cd[:])
            mw1t = wp.tile([NI, NH], bf, tag="mw1t", name="mw1t")
            nc.gpsimd.dma_start(out=mw1t, in_=mw1td[:])
            ow1t = wp.tile([128, NH], bf, tag="ow1t", name="ow1t")
            nc.gpsimd.memset(ow1t, 0.0)
            nc.gpsimd.dma_start(out=ow1t[64:128, :], in_=ow1td[:])
            mw1s = load_w_split(mw1sd, "mw1s", [nc.sync, nc.scalar])
            ow1s = load_w_split(ow1sd, "ow1s", [nc.sync, nc.scalar])
            mw2 = load_w_split(mw2d, "mw2", [nc.sync, nc.scalar])
            mw3 = load_w_split(mw3d, "mw3", [nc.sync, nc.scalar])
            ow2 = load_w_split(ow2d, "ow2", [nc.sync, nc.scalar])

            ones = smp.tile([128, 1], bf, tag="ones", name="ones")
            nc.vector.memset(ones, 1.0)
            zero = smp.tile([128, RB], bf, tag="zero", name="zero")
            nc.gpsimd.memset(zero, 0.0)

            # --- summary double buffer, init = broadcast of aggregate ---
            sA = [[sp.tile([128, RB], bf, tag=f"sA{i}_{r}",
                           name=f"sA{i}_{r}") for r in range(NR)]
                  for i in range(FT)]
            sB = [[sp.tile([128, RB], bf, tag=f"sB{i}_{r}",
                           name=f"sB{i}_{r}") for r in range(NR)]
                  for i in range(FT)]
            for r in range(NR):          # r0 first: layer1 r0 starts sooner
                for i in range(FT):
                    nc.vector.tensor_scalar(
                        sA[i][r][:], zero[:], aggt[:, ds(i, 1)], None, Add)

            # --- product accumulators ---
            pacc = []
            for r in range(NR):
                t = smp.tile([1, RB], f32, tag=f"pacc{r}", name=f"pacc{r}")
                nc.vector.memset(t, 1.0)
                pacc.append(t)

            # bias column index per layer: 0=Mb1 1=Mb2 2=Mb3 3=Ob1 4=Ob2
            def relu_epilogue(ot, ps, bias_l, m):
                """Bias+relu out of PSUM; alternate ACT/DVE by m so neither
                engine head-of-line-blocks the PE's psum bank rotation."""
                bias = ball[:, ds(bias_l * 8 + m, 1)]
                if m % 2 == 0:
                    nc.scalar.activation(ot[:], ps[:], Relu, bias=bias)
                else:
                    nc.vector.tensor_scalar(ot[:], ps[:], bias, 0.0, Add,
                                            mybir.AluOpType.max)

            def layer1(scur, tow_t, fused=True):
                """Fused M/O layer 1, two m-columns per batch.  All full-row
                summary matmuls for the four accumulation groups (M/O x
                m/m+1) run first; the four contraction-64 tower matmuls
                close the groups at the end, with M on PE rows 0-63 and O
                on rows 64-127 so each M/O pair streams concurrently and
                the full-row<->partial-row LDWEIGHTS exposure is amortized
                over two iterations.  fused=False (step 0) runs the M
                branch alone first so only its weights gate the start."""
                m1o = [[None] * FT for _ in range(NR)]
                h1o = [[None] * FT for _ in range(NR)]
                branches = ("mo",) if fused else ("m", "o")
                for br in branches:
                    for r in range(NR):
                        for mp in range(0, FT, 2):
                            psms, psos = [], []
                            for m in (mp, mp + 1):
                                if "m" in br:
                                    psm = pp.tile([128, RB],
                                                  mybir.dt.float32,
                                                  tag="ps", name="psm")
                                    psms.append(psm)
                                    for i in range(FT):
                                        nc.tensor.matmul(
                                            psm[:], mw1s[i][:, ts(m, 128)],
                                            scur[i][r][:],
                                            start=(i == 0), stop=False)
                                if "o" in br:
                                    pso = pp.tile([128, RB],
                                                  mybir.dt.float32,
                                                  tag="ps", name="pso")
                                    psos.append(pso)
                                    for i in range(FT):
                                        nc.tensor.matmul(
                                            pso[:], ow1s[i][:, ts(m, 128)],
                                            scur[i][r][:],
                                            start=(i == 0), stop=False)
                            for j, m in enumerate((mp, mp + 1)):
                                if "m" in br:
                                    nc.tensor.matmul(
                                        psms[j][:], mw1t[:, ts(m, 128)],
                                        tow_t[0:NI, ts(r, RB)],
                                        start=False, stop=True)
                                if "o" in br:
                                    nc.tensor.matmul(
                                        psos[j][:], ow1t[64:128, ts(m, 128)],
                                        tow_t[64:128, ts(r, RB)],
                                        start=False, stop=True)
                            for j, m in enumerate((mp, mp + 1)):
                                if "m" in br:
                                    m1t = ap.tile([128, RB], bf, tag="m1",
                                                  name="m1")
                                    relu_epilogue(m1t, psms[j], 0, m)
                                    m1o[r][m] = m1t
                                if "o" in br:
                                    h1t = ap.tile([128, RB], bf, tag="h1",
                                                  name="h1")
                                    relu_epilogue(h1t, psos[j], 3, m)
                                    h1o[r][m] = h1t
                reidx = lambda o: [[o[r][m] for r in range(NR)]
                                   for m in range(FT)]
                return reidx(m1o), reidx(h1o)

            def layer(rhs, ws, bias_l, out_tag, out_tiles=None):
                outs = []
                for r in range(NR):
                    row = []
                    for m in range(FT):
                        ps = pp.tile([128, RB], mybir.dt.float32, tag="ps",
                                     name="ps")
                        for i in range(FT):
                            nc.tensor.matmul(
                                ps[:], ws[i][:, ts(m, 128)], rhs[i][r][:],
                                start=(i == 0), stop=(i == FT - 1))
                        if out_tiles is not None:
                            ot = out_tiles[m][r]
                        else:
                            ot = ap.tile([128, RB], bf, tag=out_tag,
                                         name=out_tag)
                        relu_epilogue(ot, ps, bias_l, m)
                        row.append(ot)
                    outs.append(row)
                return [[outs[r][m] for r in range(NR)] for m in range(FT)]

            def flush_zjobs(zjobs):
                for gb, r in zjobs:
                    zps = zp.tile([1, RB], mybir.dt.float32, tag="z",
                                  name="zps")
                    nc.tensor.matmul(zps[:], ones[:], gb[:],
                                     start=True, stop=True)
                    pr = smp.tile([1, RB], mybir.dt.float32, tag=f"pr{r}",
                                  name=f"pr{r}")
                    nc.scalar.activation(pr[:], zps[:], Sigmoid, bias=ob3[:])
                    nc.vector.tensor_mul(pacc[r][:], pacc[r][:], pr[:])

            scur, snxt = sA, sB
            zjobs = []
            for k in range(K):
                tow_t = twp.tile([128, R], bf, tag="tow", name="tow")
                nc.gpsimd.dma_start(out=tow_t[0:NI, :], in_=towd[k])
                nc.gpsimd.dma_start(out=tow_t[64:128, :], in_=towd[k])

                m1, h1 = layer1(scur, tow_t, fused=(k != 0))
                m2 = layer(m1, mw2, 1, "l2")
                # previous step's output head (its DVE reduce is long done,
                # so the sigmoid never head-of-line-blocks the ACT queue)
                flush_zjobs(zjobs)
                zjobs = []
                layer(m2, mw3, 2, None, out_tiles=snxt)
                h2 = layer(h1, ow2, 4, "l2")
                # g = sum_i h2_i * w3_i on the DVE (per-partition scalars),
                # reduced across partitions next step by a ones-matmul.
                for r in range(NR):
                    g = zw.tile([128, RB], mybir.dt.float32, tag="g",
                                name="g")
                    nc.vector.tensor_scalar(
                        g[:], h2[0][r][:], w3c[:, ds(0, 1)], None, Mult)
                    for i in range(1, FT):
                        t = zw.tile([128, RB], mybir.dt.float32, tag="t",
                                    name="t")
                        nc.vector.tensor_scalar(
                            t[:], h2[i][r][:], w3c[:, ds(i, 1)], None, Mult)
                        nc.vector.tensor_tensor(g[:], g[:], t[:], Add)
                    gb = zw.tile([128, RB], bf, tag="gb", name="gb", bufs=4)
                    nc.vector.tensor_copy(gb[:], g[:])
                    zjobs.append((gb, r))

                scur, snxt = snxt, scur
            flush_zjobs(zjobs)

            for r in range(NR):
                nc.sync.dma_start(out=outd[:, ts(r, RB)], in_=pacc[r][:])

    nc.finalize()
    return nc


def _get_nc():
    global _BUILT
    if _BUILT is None:
        _BUILT = _build()
    return _BUILT


def _prep_inputs(inputs):
    f32 = np.float32
    towers = np.asarray(inputs["towers"], dtype=f32)
    agg = np.asarray(inputs["aggregate"], dtype=f32)
    MW1 = np.asarray(inputs["MW1"], dtype=f32)
    OW1 = np.asarray(inputs["OW1"], dtype=f32)

    shared = {
        "agg": np.ascontiguousarray(agg.reshape(FT, 128).T),
        "mw1s": MW1[:NH].astype(BF16),
        "mw1t": np.ascontiguousarray(MW1[NH:]).astype(BF16),
        "mw2": np.asarray(inputs["MW2"], f32).astype(BF16),
        "mw3": np.asarray(inputs["MW3"], f32).astype(BF16),
        "ow1s": OW1[:NH].astype(BF16),
        "ow1t": np.ascontiguousarray(OW1[NH:]).astype(BF16),
        "ow2": np.asarray(inputs["OW2"], f32).astype(BF16),
        "w3c": np.ascontiguousarray(
            np.asarray(inputs["OW3"], f32).reshape(FT, 128).T),
        "ball": np.concatenate(
            [np.asarray(inputs[b], f32).reshape(FT, 128).T
             for b in ("Mb1", "Mb2", "Mb3", "Ob1", "Ob2")], axis=1),
        "ob3": np.asarray(inputs["Ob3"], f32).reshape(1, 1),
    }
    in_maps = []
    for c in range(N_CORES):
        tc_ = towers[c * R:(c + 1) * R]          # (R, K, NI)
        towT = np.ascontiguousarray(tc_.transpose(1, 2, 0)).astype(BF16)
        in_maps.append({"tow": towT, **shared})
    return in_maps


def _run(inputs, trace=False):
    nc = _get_nc()
    in_maps = _prep_inputs(inputs)
    res = run_bass_kernel_spmd(nc, in_maps, list(range(N_CORES)), trace=trace)
    out = np.concatenate([res.results[c]["out"][0] for c in range(N_CORES)])
    return out.astype(np.float32), res


def kernel(**inputs):
    out, _ = _run(inputs, trace=False)
    return out
